# revision 1
# baseline (speedup 1.0000x reference)
"""Multi-head attention kernel for Trainium2 (Bass/Tile), 8 NeuronCores.

Problem: q,k,v [16, 4096, 128] fp32 -> softmax(q@k^T/sqrt(128))@v.
Sharding: BH=16 heads split 2-per-core across 8 cores (head parallel, no
cross-core comms).

Per-head dataflow (n = query index, m = key index, d = head dim = 128):
  - Q,K,V loaded with an fp32->fp16 cast folded into the SWDGE DMA.
  - PE-transpose Q,K into [d, n] / [d, m] fp16 SBUF layout.
  - mm1: S^T chunk [m_chunk=128, n_tile=512] = KT_chunk.T @ QT_slice (fp16
    in, fp32 PSUM out), two chunks staged per 1024-wide PSUM duo.
  - ACT: exp(scale*S^T) PSUM->SBUF fp16, 1024 elements per instruction
    (the measured ACT sweet spot), scale folded into the activation.
  - mm2: for each 128-query subtile accumulate over all 32 m-chunks:
    psum[n_sub=128, 129] += expT_chunk(stationary) @ [V|1](moving, fp16).
    Columns 0:128 = unnormalized O, column 128 = the softmax denominator
    (rides along for free; FD=129 matmuls measured at ~64 ns each).
  - DVE: reciprocal of column 128, tensor_scalar multiply -> O tile,
    batched 2MB DMA store per head.
"""
import sys

sys.path.insert(0, "/opt/trn_rl_repo")

from contextlib import ExitStack

import numpy as np

import concourse.bass as bass
import concourse.mybir as mybir
import concourse.tile as tile
from concourse import bacc
from concourse.bass_utils import run_bass_kernel_spmd
from concourse.masks import make_identity

N_CORES = 8
H_PER_CORE = 2  # BH=16 / 8 cores
N = 4096  # sequence length
D = 128  # head dim
SCALE = float(D) ** -0.5

NT = N // 128  # 32 key chunks of 128
N_TILE = 512  # query tile width for mm1
N_NTILES = N // N_TILE  # 8
DUO = 1024  # psum staging width for ACT (2 m-chunks)

F32 = mybir.dt.float32
F16 = mybir.dt.float16
EXP = mybir.ActivationFunctionType.Exp


def build_nc():
    nc = bacc.Bacc("TRN2", target_bir_lowering=False, debug=False)
    q_d = nc.dram_tensor("q", [H_PER_CORE, N, D], F32, kind="ExternalInput").ap()
    k_d = nc.dram_tensor("k", [H_PER_CORE, N, D], F32, kind="ExternalInput").ap()
    v_d = nc.dram_tensor("v", [H_PER_CORE, N, D], F32, kind="ExternalInput").ap()
    o_d = nc.dram_tensor("out", [H_PER_CORE, N, D], F32, kind="ExternalOutput").ap()

    with tile.TileContext(nc) as tc, ExitStack() as ctx:
        nat = ctx.enter_context(tc.tile_pool(name="nat", bufs=4))
        qt_p = ctx.enter_context(tc.tile_pool(name="qt", bufs=2))
        kt_p = ctx.enter_context(tc.tile_pool(name="kt", bufs=2))
        vp_p = ctx.enter_context(tc.tile_pool(name="vp", bufs=2))
        exp_p = ctx.enter_context(tc.tile_pool(name="exp", bufs=2))
        osb_p = ctx.enter_context(tc.tile_pool(name="osb", bufs=2))
        small = ctx.enter_context(tc.tile_pool(name="small", bufs=8))
        const_p = ctx.enter_context(tc.tile_pool(name="const", bufs=1))
        ps1 = ctx.enter_context(tc.tile_pool(name="ps1", bufs=2, space="PSUM"))
        ps2 = ctx.enter_context(tc.tile_pool(name="ps2", bufs=2, space="PSUM"))
        pst = ctx.enter_context(tc.tile_pool(name="pst", bufs=2, space="PSUM"))

        ident16 = const_p.tile([128, 128], F16)
        make_identity(nc, ident16[:])

        # Warm-up during the initial DMA wait: ~3.5us of dummy matmuls takes
        # the PE HAM clock gate to 2.4 GHz, and one dummy exp pre-loads the
        # ACT spline table, before the first real tiles arrive.
        warm = const_p.tile([128, 512], F16)
        nc.gpsimd.memset(warm[:], 1.0)
        wsb = const_p.tile([128, 1], F16)
        for i in range(16):
            pw = ps1.tile([128, DUO], F32, tag="ps1")
            nc.tensor.matmul(
                pw[:, 0:512], ident16[:], warm[:], start=True, stop=True
            )
            if i == 0:
                nc.scalar.activation(wsb[:], pw[:, 0:1], EXP)

        nats = {}

        def load_head(h):
            # fp32 -> fp16 cast folded into the SWDGE DMA.
            for name, src in (("q", q_d), ("k", k_d)):
                t = nat.tile([128, NT * 128], F16, tag="nat")
                nc.gpsimd.dma_start(
                    t[:].rearrange("p (t d) -> p t d", t=NT),
                    src[h].rearrange("(t p) d -> p t d", p=128),
                )
                nats[(h, name)] = t
            # V goes straight into its [V|1]-augmented home.
            vplus = vp_p.tile([128, NT * 129], F16, tag="vp")
            vp3 = vplus[:].rearrange("p (t c) -> p t c", c=129)
            nc.gpsimd.dma_start(
                vp3[:, :, 0:128],
                v_d[h].rearrange("(t p) d -> p t d", p=128),
            )
            nc.gpsimd.memset(vp3[:, :, 128:129], 1.0)
            nats[(h, "v")] = vplus

        load_head(0)

        def make_transp_ops(h):
            # One closure per 128-row tile pair; PE-transpose Q,K into the
            # [d, seq] fp16 layout.
            q_nat = nats.pop((h, "q"))
            k_nat = nats.pop((h, "k"))
            qt = qt_p.tile([128, N], F16, tag="qt")
            kt = kt_p.tile([128, N], F16, tag="kt")

            def op(t):
                sl = slice(t * 128, (t + 1) * 128)
                pq = pst.tile([128, 128], F16, tag="pst")
                nc.tensor.transpose(pq[:], q_nat[:, sl], ident16[:])
                nc.vector.tensor_copy(qt[:, sl], pq[:])
                pk = pst.tile([128, 128], F16, tag="pst")
                nc.tensor.transpose(pk[:], k_nat[:, sl], ident16[:])
                nc.vector.tensor_copy(kt[:, sl], pk[:])

            return qt, kt, [lambda t=t: op(t) for t in range(NT)]

        tqkt = {0: make_transp_ops(0)}
        for t_op in tqkt[0][2]:
            t_op()
        tqkt[0] = (tqkt[0][0], tqkt[0][1], [])

        for h in range(H_PER_CORE):
            qt, kt, _ = tqkt.pop(h)
            vplus = nats.pop((h, "v"))

            pending_transp = []
            if h + 1 < H_PER_CORE:
                load_head(h + 1)  # prefetch next head while computing
                tqkt[h + 1] = make_transp_ops(h + 1)
                pending_transp = tqkt[h + 1][2]

            osb = osb_p.tile([128, NT * 128], F32, tag="osb")

            def emit_mm2(nt, qs, expt):
                po = ps2.tile([128, 129], F32, tag="ps2")
                for mc in range(NT):
                    base = mc * N_TILE + qs * 128
                    nc.tensor.matmul(
                        po[:],
                        expt[:, base : base + 128],
                        vplus[:, mc * 129 : (mc + 1) * 129],
                        start=(mc == 0),
                        stop=(mc == NT - 1),
                    )
                rcp = small.tile([128, 1], F32, tag="rcp")
                nc.vector.reciprocal(rcp[:], po[:, 128:129])
                oc = (nt * (N_TILE // 128) + qs) * 128
                nc.vector.tensor_scalar_mul(
                    osb[:, oc : oc + 128], po[:, 0:128], rcp[:]
                )

            prev = None
            for nt in range(N_NTILES):
                qsl = slice(nt * N_TILE, (nt + 1) * N_TILE)
                expt = exp_p.tile([128, NT * N_TILE], F16, tag="exp")
                for duo in range(NT // 2):
                    ps = ps1.tile([128, DUO], F32, tag="ps1")
                    for j in range(2):
                        mc = duo * 2 + j
                        nc.tensor.matmul(
                            ps[:, j * N_TILE : (j + 1) * N_TILE],
                            kt[:, mc * 128 : (mc + 1) * 128],
                            qt[:, qsl],
                            start=True,
                            stop=True,
                        )
                    nc.scalar.activation(
                        expt[:, duo * DUO : (duo + 1) * DUO],
                        ps[:],
                        EXP,
                        scale=SCALE,
                    )
                    if prev is not None and duo % 4 == 3:
                        emit_mm2(prev[0], duo // 4, prev[1])
                    if nt == N_NTILES - 1 and pending_transp:
                        # Slip the next head's Q/K transposes into the last
                        # n-tile's stream so the head switch has no PE ramp.
                        pending_transp.pop(0)()
                        pending_transp.pop(0)()
                prev = (nt, expt)
            if pending_transp:
                for t_op in pending_transp:
                    t_op()
            if h + 1 < H_PER_CORE:
                tqkt[h + 1] = (tqkt[h + 1][0], tqkt[h + 1][1], [])
            for qs in range(N_TILE // 128):
                emit_mm2(prev[0], qs, prev[1])

            nc.sync.dma_start(
                o_d[h].rearrange("(t p) d -> p t d", p=128),
                osb[:].rearrange("p (t d) -> p t d", t=NT),
            )

    nc.finalize()
    return nc


_NC_CACHE = None


def _get_nc():
    global _NC_CACHE
    if _NC_CACHE is None:
        _NC_CACHE = build_nc()
    return _NC_CACHE


def run(q, k, v, **spmd_kwargs):
    nc = _get_nc()
    in_maps = [
        {
            "q": np.ascontiguousarray(q[i * H_PER_CORE : (i + 1) * H_PER_CORE]),
            "k": np.ascontiguousarray(k[i * H_PER_CORE : (i + 1) * H_PER_CORE]),
            "v": np.ascontiguousarray(v[i * H_PER_CORE : (i + 1) * H_PER_CORE]),
        }
        for i in range(N_CORES)
    ]
    last_err = None
    for _ in range(3):  # retry transient NRT execution errors
        try:
            res = run_bass_kernel_spmd(
                nc, in_maps, list(range(N_CORES)), **spmd_kwargs
            )
            break
        except Exception as e:  # noqa: BLE001
            last_err = e
    else:
        raise last_err
    out = np.concatenate([res.results[i]["out"] for i in range(N_CORES)], axis=0)
    return out.astype(np.float32), res


def kernel(q, k, v):
    q = np.asarray(q, dtype=np.float32)
    k = np.asarray(k, dtype=np.float32)
    v = np.asarray(v, dtype=np.float32)
    out, _ = run(q, k, v)
    return out



# revision 7
# speedup vs baseline: 1.0348x; 1.0348x over previous
"""Multi-head attention kernel for Trainium2 (Bass/Tile), 8 NeuronCores.

Problem: q,k,v [16, 4096, 128] fp32 -> softmax(q@k^T/sqrt(128))@v.
Sharding: BH=16 heads split 2-per-core across 8 cores (head parallel, no
cross-core comms).

Per-head dataflow (n = query index, m = key index, d = head dim = 128):
  - Q,K,V loaded with an fp32->fp16 cast folded into the SWDGE DMA.
  - PE-transpose Q,K into [d, n] / [d, m] fp16 SBUF layout.
  - mm1: S^T chunk [m_chunk=128, n_tile=512] = KT_chunk.T @ QT_slice (fp16
    in, fp32 PSUM out), two chunks staged per 1024-wide PSUM duo.
  - exp(scale*S^T) PSUM->SBUF fp16 split across two engines: 11 of 16
    duos on ACT (exact spline exp, 1024 elements per instruction), 5 on
    DVE via the Schraudolph bit trick (y = A*s + B as fp16, convert to
    int16, bitcast back to fp16 = 2^(log2e*scale*s) with ~1.8% rms
    elementwise error on 5/16 of the scores -> ~1e-2 output rel err).
    The ACT engine alone (1 elem/cycle/lane @ 1.2 GHz) is the kernel
    bottleneck at ~266 us/core; the split brings both pipes under the
    PE fp16 floor (~226 us/core).
  - mm2: for each 128-query subtile accumulate over all 32 m-chunks:
    psum[n_sub=128, 129] += expT_chunk(stationary) @ [V|1](moving, fp16).
    Columns 0:128 = unnormalized O, column 128 = the softmax denominator
    (rides along for free; FD=129 matmuls measured at ~64 ns each).
  - DVE: reciprocal of column 128, tensor_scalar multiply -> O tile,
    batched 2MB DMA store per head.
"""
import sys

sys.path.insert(0, "/opt/trn_rl_repo")

from contextlib import ExitStack

import numpy as np

import concourse.bass as bass
import concourse.mybir as mybir
import concourse.tile as tile
from concourse import bacc
from concourse.bass_utils import run_bass_kernel_spmd
from concourse.masks import make_identity

N_CORES = 8
H_PER_CORE = 2  # BH=16 / 8 cores
N = 4096  # sequence length
D = 128  # head dim
SCALE = float(D) ** -0.5

NT = N // 128  # 32 key chunks of 128
N_TILE = 512  # query tile width for mm1
N_NTILES = N // N_TILE  # 8
DUO = 1024  # psum staging width for ACT (2 m-chunks)

F32 = mybir.dt.float32
F16 = mybir.dt.float16
I16 = mybir.dt.int16
EXP = mybir.ActivationFunctionType.Exp

# Schraudolph fp16 exp2 bit trick: bits = round(A*s + B) interpreted as fp16
# gives exp(scale*s) with ~1.8% rms relative error (c=59 zeroes the mean).
A_SCH = float(1024.0 * np.log2(np.e) * SCALE)
B_SCH = float(15360.0 - 59.0)
DVE_DUOS = frozenset({3, 6, 9, 12, 15})  # 5 of 16 duos exp'd on DVE


def build_nc():
    nc = bacc.Bacc("TRN2", target_bir_lowering=False, debug=False)
    q_d = nc.dram_tensor("q", [H_PER_CORE, N, D], F32, kind="ExternalInput").ap()
    k_d = nc.dram_tensor("k", [H_PER_CORE, N, D], F32, kind="ExternalInput").ap()
    v_d = nc.dram_tensor("v", [H_PER_CORE, N, D], F32, kind="ExternalInput").ap()
    o_d = nc.dram_tensor("out", [H_PER_CORE, N, D], F32, kind="ExternalOutput").ap()

    with tile.TileContext(nc) as tc, ExitStack() as ctx:
        nat = ctx.enter_context(tc.tile_pool(name="nat", bufs=4))
        qt_p = ctx.enter_context(tc.tile_pool(name="qt", bufs=2))
        kt_p = ctx.enter_context(tc.tile_pool(name="kt", bufs=2))
        vp_p = ctx.enter_context(tc.tile_pool(name="vp", bufs=2))
        exp_p = ctx.enter_context(tc.tile_pool(name="exp", bufs=2))
        y_p = ctx.enter_context(tc.tile_pool(name="ysch", bufs=2))
        osb_p = ctx.enter_context(tc.tile_pool(name="osb", bufs=2))
        small = ctx.enter_context(tc.tile_pool(name="small", bufs=8))
        const_p = ctx.enter_context(tc.tile_pool(name="const", bufs=1))
        ps1 = ctx.enter_context(tc.tile_pool(name="ps1", bufs=2, space="PSUM"))
        ps2 = ctx.enter_context(tc.tile_pool(name="ps2", bufs=2, space="PSUM"))
        pst = ctx.enter_context(tc.tile_pool(name="pst", bufs=2, space="PSUM"))

        ident16 = const_p.tile([128, 128], F16)
        make_identity(nc, ident16[:])

        # Warm-up during the initial DMA wait: ~3.5us of dummy matmuls takes
        # the PE HAM clock gate to 2.4 GHz, and one dummy exp pre-loads the
        # ACT spline table, before the first real tiles arrive.
        warm = const_p.tile([128, 512], F16)
        nc.gpsimd.memset(warm[:], 1.0)
        wsb = const_p.tile([128, 1], F16)
        for i in range(16):
            pw = ps1.tile([128, DUO], F32, tag="ps1")
            nc.tensor.matmul(
                pw[:, 0:512], ident16[:], warm[:], start=True, stop=True
            )
            if i == 0:
                nc.scalar.activation(wsb[:], pw[:, 0:1], EXP)

        nats = {}

        def load_head(h):
            # fp32 -> fp16 cast folded into the SWDGE DMA.
            for name, src in (("q", q_d), ("k", k_d)):
                t = nat.tile([128, NT * 128], F16, tag="nat")
                nc.gpsimd.dma_start(
                    t[:].rearrange("p (t d) -> p t d", t=NT),
                    src[h].rearrange("(t p) d -> p t d", p=128),
                )
                nats[(h, name)] = t
            # V goes straight into its [V|1]-augmented home.
            vplus = vp_p.tile([128, NT * 129], F16, tag="vp")
            vp3 = vplus[:].rearrange("p (t c) -> p t c", c=129)
            nc.gpsimd.dma_start(
                vp3[:, :, 0:128],
                v_d[h].rearrange("(t p) d -> p t d", p=128),
            )
            nc.gpsimd.memset(vp3[:, :, 128:129], 1.0)
            nats[(h, "v")] = vplus

        load_head(0)

        def make_transp_ops(h):
            # One closure per 128-row tile pair; PE-transpose Q,K into the
            # [d, seq] fp16 layout.
            q_nat = nats.pop((h, "q"))
            k_nat = nats.pop((h, "k"))
            qt = qt_p.tile([128, N], F16, tag="qt")
            kt = kt_p.tile([128, N], F16, tag="kt")

            def op(t):
                sl = slice(t * 128, (t + 1) * 128)
                pq = pst.tile([128, 128], F16, tag="pst")
                nc.tensor.transpose(pq[:], q_nat[:, sl], ident16[:])
                nc.vector.tensor_copy(qt[:, sl], pq[:])
                pk = pst.tile([128, 128], F16, tag="pst")
                nc.tensor.transpose(pk[:], k_nat[:, sl], ident16[:])
                nc.vector.tensor_copy(kt[:, sl], pk[:])

            return qt, kt, [lambda t=t: op(t) for t in range(NT)]

        tqkt = {0: make_transp_ops(0)}
        for t_op in tqkt[0][2]:
            t_op()
        tqkt[0] = (tqkt[0][0], tqkt[0][1], [])

        for h in range(H_PER_CORE):
            qt, kt, _ = tqkt.pop(h)
            vplus = nats.pop((h, "v"))

            pending_transp = []
            if h + 1 < H_PER_CORE:
                load_head(h + 1)  # prefetch next head while computing
                tqkt[h + 1] = make_transp_ops(h + 1)
                pending_transp = tqkt[h + 1][2]

            osb = osb_p.tile([128, NT * 128], F32, tag="osb")

            def emit_mm2(nt, qs, expt):
                po = ps2.tile([128, 129], F32, tag="ps2")
                for mc in range(NT):
                    base = mc * N_TILE + qs * 128
                    nc.tensor.matmul(
                        po[:],
                        expt[:, base : base + 128],
                        vplus[:, mc * 129 : (mc + 1) * 129],
                        start=(mc == 0),
                        stop=(mc == NT - 1),
                    )
                rcp = small.tile([128, 1], F32, tag="rcp")
                nc.vector.reciprocal(rcp[:], po[:, 128:129])
                oc = (nt * (N_TILE // 128) + qs) * 128
                nc.vector.tensor_scalar_mul(
                    osb[:, oc : oc + 128], po[:, 0:128], rcp[:]
                )

            prev = None
            for nt in range(N_NTILES):
                qsl = slice(nt * N_TILE, (nt + 1) * N_TILE)
                expt = exp_p.tile([128, NT * N_TILE], F16, tag="exp")
                for duo in range(NT // 2):
                    ps = ps1.tile([128, DUO], F32, tag="ps1")
                    for j in range(2):
                        mc = duo * 2 + j
                        nc.tensor.matmul(
                            ps[:, j * N_TILE : (j + 1) * N_TILE],
                            kt[:, mc * 128 : (mc + 1) * 128],
                            qt[:, qsl],
                            start=True,
                            stop=True,
                        )
                    exp_sl = expt[:, duo * DUO : (duo + 1) * DUO]
                    if duo in DVE_DUOS:
                        y16 = y_p.tile([128, DUO], F16, tag="ysch")
                        nc.vector.tensor_scalar(
                            y16[:],
                            ps[:],
                            A_SCH,
                            B_SCH,
                            mybir.AluOpType.mult,
                            mybir.AluOpType.add,
                        )
                        nc.vector.tensor_copy(exp_sl.bitcast(I16), y16[:])
                    else:
                        nc.scalar.activation(exp_sl, ps[:], EXP, scale=SCALE)
                    if prev is not None and duo % 4 == 3:
                        emit_mm2(prev[0], duo // 4, prev[1])
                    if nt == N_NTILES - 1 and pending_transp:
                        # Slip the next head's Q/K transposes into the last
                        # n-tile's stream so the head switch has no PE ramp.
                        pending_transp.pop(0)()
                        pending_transp.pop(0)()
                prev = (nt, expt)
            if pending_transp:
                for t_op in pending_transp:
                    t_op()
            if h + 1 < H_PER_CORE:
                tqkt[h + 1] = (tqkt[h + 1][0], tqkt[h + 1][1], [])
            for qs in range(N_TILE // 128):
                emit_mm2(prev[0], qs, prev[1])

            nc.sync.dma_start(
                o_d[h].rearrange("(t p) d -> p t d", p=128),
                osb[:].rearrange("p (t d) -> p t d", t=NT),
            )

    nc.finalize()
    return nc


_NC_CACHE = None


def _get_nc():
    global _NC_CACHE
    if _NC_CACHE is None:
        _NC_CACHE = build_nc()
    return _NC_CACHE


def run(q, k, v, **spmd_kwargs):
    nc = _get_nc()
    in_maps = [
        {
            "q": np.ascontiguousarray(q[i * H_PER_CORE : (i + 1) * H_PER_CORE]),
            "k": np.ascontiguousarray(k[i * H_PER_CORE : (i + 1) * H_PER_CORE]),
            "v": np.ascontiguousarray(v[i * H_PER_CORE : (i + 1) * H_PER_CORE]),
        }
        for i in range(N_CORES)
    ]
    last_err = None
    for _ in range(3):  # retry transient NRT execution errors
        try:
            res = run_bass_kernel_spmd(
                nc, in_maps, list(range(N_CORES)), **spmd_kwargs
            )
            break
        except Exception as e:  # noqa: BLE001
            last_err = e
    else:
        raise last_err
    out = np.concatenate([res.results[i]["out"] for i in range(N_CORES)], axis=0)
    return out.astype(np.float32), res


def kernel(q, k, v):
    q = np.asarray(q, dtype=np.float32)
    k = np.asarray(k, dtype=np.float32)
    v = np.asarray(v, dtype=np.float32)
    out, _ = run(q, k, v)
    return out

